# revision 1
# baseline (speedup 1.0000x reference)
"""ARMA GNN (2-layer, K=2 stacks) Trainium2 kernel.

Strategy (8-core SPMD, node-sharded), v7 "direct-table AllGather + flip":
  * norm folding: norm[e] = dinv[row]*dinv[col]; aggregation is linear, so
    each layer aggregates RAW scaled features and applies the weight matrix
    per 128-node window afterwards:
        xagg[n] = sum_{e: col=n} (dinv[row]*feat[row])
        out_k   = relu(0.5*(dinv[n]*(xagg @ Wk) + feat@RWk + bk)); mean_k
  * Gather-table rows live in a chunk-major permuted row space so chunked
    AllGathers (issued mid-layer-1, two batches after each chunk's windows
    complete) land contiguously into ccout, which layer 2 gathers as-is;
    the layer-1 table (dinv*x fp16) is host-built in the same space, and
    both layers share the same token/selection streams.
  * Layer-1 epilogue writes dinv*h1 into the low half of 128-wide padded
    ccpad rows; no post-collective table build exists.
  * Padding tokens point at row 0 with slot value -1 (selection column is
    all-zero), so no zero rows are needed anywhere.
  * Flipped aggregation matmul: lhsT = gathered 128-edge block M[e,f]
    (stationary), rhs = selection matrix S[e,slot] = is_equal(iota, col)
    built on DVE -> PSUM agg.T[f,slot].  No per-window PE transpose; agg.T
    feeds the Wcat matmul's lhsT directly after one PSUM->SBUF copy.
  * Edges sharded by target; per 128-edge block one matmul accumulated in
    PSUM; dma_gather on 4 SWDGE queues round-robin, two calls per batch.
  * Root/bias adds are batched 4 windows per DVE op ([P,512]); root2 is
    built from local h1 while the trailing AllGather chunk runs.

kernel(**inputs) takes the FULL problem inputs and returns the FULL output.
"""

import sys

sys.path.insert(0, "/opt/trn_rl_repo")

from contextlib import ExitStack

import numpy as np

P = 128


class Cfg:
    def __init__(self, N, NC, SHARD, B0, WB=2, G=8, SUPER=4,
                 FIN=128, HID=64, FOUT=64, K=2, SP=False):
        self.N, self.NC, self.SHARD, self.B0 = N, NC, SHARD, B0
        self.WB, self.G, self.SUPER = WB, G, SUPER
        self.FIN, self.HID, self.FOUT, self.K = FIN, HID, FOUT, K
        self.SP = SP
        self.NSTAR = NC * SHARD
        self.W = SHARD // P
        self.HALFA = B0
        self.HALFB = self.NSTAR - B0
        assert B0 % P == 0 and SHARD % P == 0
        assert self.HALFA <= 32768 and self.HALFB <= 32768
        assert N > B0 and N < self.NSTAR
        assert K * HID == 128 and K * FOUT == 128 and FIN == 128
        self.batches = [tuple(range(b, min(b + WB, self.W)))
                        for b in range(0, self.W, WB)]
        self.sbatches = [self.batches[i:i + SUPER]
                         for i in range(0, len(self.batches), SUPER)]
        # collective chunks with a tapered tail so the last exchange is small
        sizes = []
        rem = self.W
        for f in (0.30, 0.29, 0.25, 0.10, 0.06):
            sz = max(1, min(rem - 1, round(self.W * f))) if rem > 1 else 0
            if sz:
                sizes.append(sz)
                rem -= sz
        if rem:
            sizes.append(rem)
        self.cc_chunks = []
        w0 = 0
        for sz in sizes:
            self.cc_chunks.append((w0, w0 + sz))
            w0 += sz
        assert w0 == self.W
        self.chunk_base = []
        b = 0
        for (w0, w1) in self.cc_chunks:
            self.chunk_base.append(b)
            b += NC * (w1 - w0) * P
        assert b == self.NSTAR

    def perm(self, node):
        """node-id -> permuted gather-table row (static, data-independent)."""
        node = np.asarray(node)
        r, l = node // self.SHARD, node % self.SHARD
        w = l // P
        prow = np.zeros_like(node)
        for q, (w0, w1) in enumerate(self.cc_chunks):
            m = (w >= w0) & (w < w1)
            prow = np.where(
                m, self.chunk_base[q] + r * (w1 - w0) * P + (l - w0 * P), prow)
        return prow


REAL = dict(N=50000, NC=8, SHARD=6272, B0=25088)


# --------------------------------------------------------------------------
# host preprocessing
# --------------------------------------------------------------------------
def _preprocess(c: Cfg, x, edge_index, init_w1, root_w1, b1, init_w2, root_w2, b2):
    N, NC, SHARD = c.N, c.NC, c.SHARD
    row = np.asarray(edge_index[0]).astype(np.int64)
    col = np.asarray(edge_index[1]).astype(np.int64)
    x = np.asarray(x, dtype=np.float32)

    deg = np.bincount(col, minlength=N).astype(np.float64)
    dinv = np.where(deg > 0, deg ** -0.5, 0.0).astype(np.float32)
    dinv_full = np.zeros(c.NSTAR, np.float32)
    dinv_full[:N] = dinv

    prow_all = c.perm(row)

    percore = []
    counts = np.zeros((NC, c.W, 2), np.int64)
    for cc in range(NC):
        base = cc * SHARD
        m = (col >= base) & (col < base + SHARD)
        ec = (col[m] - base).astype(np.int64)
        es = prow_all[m]
        half = (es >= c.B0).astype(np.int64)
        key = (ec >> 7) * 2 + half
        order = np.argsort(key, kind="stable")
        ec, es, key = ec[order], es[order], key[order]
        bounds = np.searchsorted(key, np.arange(2 * c.W + 1))
        percore.append((ec, es, bounds))
        for w in range(c.W):
            counts[cc, w, 0] = bounds[2 * w + 1] - bounds[2 * w]
            counts[cc, w, 1] = bounds[2 * w + 2] - bounds[2 * w + 1]

    NBA = [max(1, int(-(-counts[:, w, 0].max() // P))) for w in range(c.W)]
    NBB = [max(1, int(-(-counts[:, w, 1].max() // P))) for w in range(c.W)]

    def build_stream(cc, half_id, NB_list):
        ec, es, bounds = percore[cc]
        toks, slots = [], []
        for batch in c.batches:
            for w in batch:
                lo, hi = bounds[2 * w + half_id], bounds[2 * w + half_id + 1]
                k = hi - lo
                n = NB_list[w] * P
                t = np.zeros(n, np.int64)          # pads gather row 0
                s = np.full(n, -1.0, np.float32)   # pads select nothing
                t[:k] = es[lo:hi] - (c.B0 if half_id else 0)
                s[:k] = ec[lo:hi] & 127
                toks.append(t)
                slots.append(s)
        toks = np.concatenate(toks)
        slots = np.concatenate(slots).astype(np.float16)
        L = len(toks)
        idxw = np.tile(toks.reshape(L // 16, 16).T.astype(np.int16), (8, 1))
        colf = slots.reshape(L // P, P).T  # [128, NB] f16
        return np.ascontiguousarray(idxw), np.ascontiguousarray(colf)

    # layer-1 gather table: dinv*x in permuted row order, fp16
    xs = np.zeros((c.NSTAR, c.FIN), np.float32)
    xs[c.perm(np.arange(N))] = x * dinv[:, None]
    xs16 = np.ascontiguousarray(xs.astype(np.float16))

    xpad = np.zeros((c.NSTAR, c.FIN), np.float32)
    xpad[:N] = x

    def cat2(w, dt):
        w = np.asarray(w, dtype=np.float32)
        return np.ascontiguousarray(np.concatenate([w[0], w[1]], axis=1).astype(dt))

    w1cat = cat2(init_w1, np.float32)            # [128,128] f32
    w2cat = cat2(init_w2, np.float32)            # [64,128]  f32
    rw1c = cat2(0.5 * np.asarray(root_w1, np.float32), np.float16)
    rw2c = cat2(0.5 * np.asarray(root_w2, np.float32), np.float32)
    b1 = np.asarray(b1, dtype=np.float32)
    b2 = np.asarray(b2, dtype=np.float32)
    b1b = np.ascontiguousarray(
        np.tile(0.5 * np.concatenate([b1[0], b1[1]]), (P, 4)))
    b2b = np.ascontiguousarray(
        np.tile(0.5 * np.concatenate([b2[0], b2[1]]), (P, 4)))

    in_maps = []
    for cc in range(NC):
        base = cc * SHARD
        idxA, colfA = build_stream(cc, 0, NBA)
        idxB, colfB = build_stream(cc, 1, NBB)
        dinvo = 0.5 * dinv_full[base:base + SHARD].reshape(c.W, P).T
        dinvt = dinv_full[base:base + SHARD].reshape(c.W, P).T
        in_maps.append({
            "xs": xs16,
            "xTow": np.ascontiguousarray(xpad[base:base + SHARD].T.astype(np.float16)),
            "w1cat": w1cat, "rw1c": rw1c, "w2cat": w2cat, "rw2c": rw2c,
            "b1b": b1b, "b2b": b2b,
            "dinvo": np.ascontiguousarray(dinvo.astype(np.float32)),
            "dinvt": np.ascontiguousarray(dinvt.astype(np.float32)),
            "idxA": idxA, "idxB": idxB,
            "colfA": colfA, "colfB": colfB,
        })
    return in_maps, NBA, NBB


# --------------------------------------------------------------------------
# device program
# --------------------------------------------------------------------------
def _build_program(c: Cfg, NBA, NBB):
    import concourse.tile as tile
    from concourse import bacc, mybir
    from concourse.masks import make_identity

    f32 = mybir.dt.float32
    f16 = mybir.dt.float16
    i16 = mybir.dt.int16
    AL = mybir.AluOpType
    AF = mybir.ActivationFunctionType

    NBAtot, NBBtot = sum(NBA), sum(NBB)
    LA, LB = NBAtot * P, NBBtot * P

    nc = bacc.Bacc("TRN2", target_bir_lowering=False, debug=False,
                   num_devices=c.NC, num_swdge_queues=4)
    qrr = [0]

    def din(name, shape, dt=f32):
        return nc.dram_tensor(name, shape, dt, kind="ExternalInput")

    xs = din("xs", [c.NSTAR, 128], f16)          # layer-1 gather table
    xTow = din("xTow", [P, c.SHARD], f16)
    w1cat = din("w1cat", [P, 128], f32)
    rw1c = din("rw1c", [P, 128], f16)
    w2cat = din("w2cat", [64, 128], f32)
    rw2c = din("rw2c", [64, 128], f32)
    b1b = din("b1b", [P, 512]); b2b = din("b2b", [P, 512])
    dinvo = din("dinvo", [P, c.W])
    dinvt = din("dinvt", [P, c.W])
    idxA = din("idxA", [P, LA // 16], i16)
    idxB = din("idxB", [P, LB // 16], i16)
    colfA = din("colfA", [P, NBAtot], f16)
    colfB = din("colfB", [P, NBBtot], f16)
    yt = nc.dram_tensor("yt", [c.SHARD, 64], f32, kind="ExternalOutput")

    ccpad = nc.dram_tensor("ccpad", [c.SHARD, 128], f16)
    ccout = nc.dram_tensor("ccout", [c.NSTAR, 128], f16)

    with tile.TileContext(nc) as tc, ExitStack() as ctx:
        cpool = ctx.enter_context(tc.tile_pool(name="consts", bufs=1))
        xtp = ctx.enter_context(tc.tile_pool(name="xtp", bufs=3))
        gth = ctx.enter_context(tc.tile_pool(name="gth", bufs=10))
        sgp = ctx.enter_context(tc.tile_pool(name="sgp", bufs=8))
        idxp = ctx.enter_context(tc.tile_pool(name="idxp", bufs=3))
        epi = ctx.enter_context(tc.tile_pool(name="epi", bufs=3))
        big = ctx.enter_context(tc.tile_pool(name="big", bufs=1))
        psA = ctx.enter_context(tc.tile_pool(name="psA", bufs=2, space="PSUM"))
        psB = ctx.enter_context(tc.tile_pool(name="psB", bufs=3, space="PSUM"))
        psC = ctx.enter_context(tc.tile_pool(name="psC", bufs=2, space="PSUM"))

        ident = cpool.tile([P, P], f32, tag="ident")
        make_identity(nc, ident[:])
        iota_i = cpool.tile([P, c.G * P], mybir.dt.int32, tag="iotai")
        nc.gpsimd.iota(iota_i[:], pattern=[[0, c.G], [1, P]], base=0,
                       channel_multiplier=0)
        iota_16 = cpool.tile([P, c.G * P], f16, tag="iota16")
        nc.vector.tensor_copy(iota_16[:], iota_i[:])

        def load_const(dram, shape, tag, dt=f32):
            t = cpool.tile(shape, dt, tag=tag)
            nc.sync.dma_start(t[:], dram[:, :])
            return t

        w1_s = load_const(w1cat, [P, 128], "w1")
        rw1_s = load_const(rw1c, [P, 128], "rw1", f16)
        w2_s = load_const(w2cat, [64, 128], "w2")
        rw2_s = load_const(rw2c, [64, 128], "rw2")
        b1_s = load_const(b1b, [P, 512], "b1")
        b2_s = load_const(b2b, [P, 512], "b2")
        dinvo_s = load_const(dinvo, [P, c.W], "dinvo")
        dinvt_s = load_const(dinvt, [P, c.W], "dinvt")
        cA16 = load_const(colfA, [P, NBAtot], "cA16", f16)
        cB16 = load_const(colfB, [P, NBBtot], "cB16", f16)

        # ---- queue warmup: tiny gather per SWDGE queue, overlaps prolog ----
        with nc.named_scope("warm"):
            wix = idxp.tile([P, 8], i16, tag="ixA")
            nc.sync.dma_start(wix[:], idxA[:, 0:8])
            for q in range(4):
                wg = gth.tile([P, 128], f16, tag="gath")
                nc.gpsimd.dma_gather(
                    out_ap=wg[:].rearrange("p (b f) -> p b f", f=128),
                    in_ap=xs[0:c.HALFA, :],
                    idxs_ap=wix[:, 0:8],
                    num_idxs=128, num_idxs_reg=128, elem_size=128,
                    single_packet=c.SP, queue_num=q)

        # ---- prolog: root1 (bias adds batched 4 windows per op) ----
        with nc.named_scope("prolog"):
            root1 = big.tile([P, c.SHARD], f32, tag="root")
            i = 0
            while i < c.W:
                n = min(8, c.W - i)
                xp = xtp.tile([P, 8 * 128], f16, tag="xtp")
                nc.sync.dma_start(xp[:, :n * 128], xTow[:, i * P:(i + n) * P])
                j = 0
                while j < n:
                    g = min(4, n - j)
                    ps = psA.tile([P, 512], f32, tag="grp")
                    for k in range(g):
                        nc.tensor.matmul(
                            out=ps[:, k * 128:(k + 1) * 128],
                            lhsT=xp[:, (j + k) * 128:(j + k + 1) * 128],
                            rhs=rw1_s[:], start=True, stop=True)
                    nc.vector.tensor_tensor(
                        out=root1[:, (i + j) * 128:(i + j + g) * 128],
                        in0=ps[:, :g * 128], in1=b1_s[:, :g * 128], op=AL.add)
                    j += g
                i += n

        # ---- generic layer: two half-streams, flipped matmul ----
        def layer(tabA_ap, tabB_ap, root_t, out_t, xw, wc_s, on_window=None):
            blkA = blkB = 0
            tokA = tokB = 0

            def build_s(n_blk, blk0, colf_s):
                tiles = []
                for g0 in range(0, n_blk, c.G):
                    gl = min(c.G, n_blk - g0)
                    s_t = sgp.tile([P, c.G * 128], f16, tag="sg")
                    nc.vector.tensor_tensor(
                        out=s_t[:, :gl * 128], in0=iota_16[:, :gl * 128],
                        in1=colf_s[:, blk0 + g0:blk0 + g0 + gl]
                            .to_broadcast([P, gl, 128]),
                        op=AL.is_equal)
                    tiles.append(s_t)
                return tiles

            for sb in c.sbatches:
                sbA = sum(NBA[w] for b in sb for w in b) * P
                sbB = sum(NBB[w] for b in sb for w in b) * P
                ixA = idxp.tile([P, sbA // 16], i16, tag="ixA")
                nc.sync.dma_start(ixA[:], idxA[:, tokA // 16:(tokA + sbA) // 16])
                ixB = idxp.tile([P, sbB // 16], i16, tag="ixB")
                nc.sync.dma_start(ixB[:], idxB[:, tokB // 16:(tokB + sbB) // 16])
                lA = lB = 0
                for batch in sb:
                    nA = sum(NBA[w] for w in batch)
                    nB = sum(NBB[w] for w in batch)
                    gA = gth.tile([P, nA * 128], f16, tag="gath")
                    nc.gpsimd.dma_gather(
                        out_ap=gA[:].rearrange("p (b f) -> p b f", f=128),
                        in_ap=tabA_ap,
                        idxs_ap=ixA[:, lA // 16:(lA + nA * P) // 16],
                        num_idxs=nA * P, num_idxs_reg=nA * P, elem_size=128,
                        single_packet=c.SP, queue_num=qrr[0] % 4)
                    qrr[0] += 1
                    gB = gth.tile([P, nB * 128], f16, tag="gath")
                    nc.gpsimd.dma_gather(
                        out_ap=gB[:].rearrange("p (b f) -> p b f", f=128),
                        in_ap=tabB_ap,
                        idxs_ap=ixB[:, lB // 16:(lB + nB * P) // 16],
                        num_idxs=nB * P, num_idxs_reg=nB * P, elem_size=128,
                        single_packet=c.SP, queue_num=qrr[0] % 4)
                    qrr[0] += 1
                    lA += nA * P
                    lB += nB * P
                    sA = build_s(nA, blkA, cA16)
                    sB_ = build_s(nB, blkB, cB16)
                    oA = oB = 0
                    for w in batch:
                        pw = psB.tile([P, 128], f32, tag="pw")
                        nmm = NBA[w] + NBB[w]
                        k = 0
                        for j in range(NBA[w]):
                            b = oA + j
                            nc.tensor.matmul(
                                out=pw[:],
                                lhsT=gA[:, b * 128:(b + 1) * 128],
                                rhs=sA[b // c.G][:, (b % c.G) * 128:(b % c.G + 1) * 128],
                                start=(k == 0), stop=(k == nmm - 1))
                            k += 1
                        for j in range(NBB[w]):
                            b = oB + j
                            nc.tensor.matmul(
                                out=pw[:],
                                lhsT=gB[:, b * 128:(b + 1) * 128],
                                rhs=sB_[b // c.G][:, (b % c.G) * 128:(b % c.G + 1) * 128],
                                start=(k == 0), stop=(k == nmm - 1))
                            k += 1
                        oA += NBA[w]; oB += NBB[w]
                        # window transform: agg.T is already lhsT-oriented
                        utc = epi.tile([P, 128], f32, tag="utc")
                        nc.scalar.copy(utc[:xw, :], pw[:xw, :])
                        pw2 = psC.tile([P, 128], f32, tag="pw2")
                        nc.tensor.matmul(out=pw2[:], lhsT=utc[:xw, :],
                                         rhs=wc_s[:], start=True, stop=True)
                        t2 = epi.tile([P, 128], f32, tag="t2")
                        nc.vector.scalar_tensor_tensor(
                            out=t2[:], in0=pw2[:], scalar=dinvo_s[:, w:w + 1],
                            in1=root_t[:, w * 128:(w + 1) * 128],
                            op0=AL.mult, op1=AL.add)
                        t3 = epi.tile([P, 128], f32, tag="t3")
                        nc.scalar.activation(t3[:], t2[:], AF.Relu)
                        nc.vector.tensor_tensor(
                            out=out_t[:, w * 64:(w + 1) * 64],
                            in0=t3[:, :64], in1=t3[:, 64:], op=AL.add)
                        if on_window is not None:
                            on_window(w)
                    blkA += nA; blkB += nB
                tokA += sbA; tokB += sbB

        # layer-1 epilogue hook: scaled-h1 store + chunked AllGather issue
        h1n = big.tile([P, c.W * 64], f16, tag="ht")
        pending = []      # (chunk_idx, issue_at_window)
        chunk_iter = iter(range(len(c.cc_chunks)))
        next_q = next(chunk_iter)

        def issue_cc(q):
            w0, w1 = c.cc_chunks[q]
            b0 = c.chunk_base[q]
            nc.gpsimd.collective_compute(
                "AllGather", AL.bypass,
                replica_groups=[list(range(c.NC))],
                ins=[ccpad[w0 * P:w1 * P, :].opt()],
                outs=[ccout[b0:b0 + c.NC * (w1 - w0) * P, :].opt()])

        def on_window1(w):
            nonlocal next_q
            while pending and w >= pending[0][1]:
                issue_cc(pending.pop(0)[0])
            sc = epi.tile([P, 128], f16, tag="sc")
            nc.vector.memset(sc[:, 64:], 0.0)
            nc.scalar.mul(sc[:, :64], h1n[:, w * 64:(w + 1) * 64],
                          dinvt_s[:, w:w + 1])
            nc.sync.dma_start(
                ccpad[w * P:(w + 1) * P, :]
                .rearrange("(k p) f -> p k f", p=P),
                sc[:].rearrange("p (k f) -> p k f", k=1))
            if next_q is not None and w == c.cc_chunks[next_q][1] - 1:
                pending.append((next_q, w + c.WB))
                next_q = next(chunk_iter, None)

        with nc.named_scope("layer1"):
            layer(xs[0:c.HALFA, :], xs[c.HALFA:c.NSTAR, :], root1, h1n, 128,
                  w1_s, on_window=on_window1)
        with nc.named_scope("cc"):
            for q, _ in pending:
                issue_cc(q)

        with nc.named_scope("mid"):
            # root2 from local h1 (overlaps the trailing collective)
            root2 = big.tile([P, c.SHARD], f32, tag="root")
            j = 0
            while j < c.W:
                g = min(4, c.W - j)
                ps = psA.tile([P, 512], f32, tag="grp")
                for k in range(g):
                    u2 = epi.tile([P, 64], f32, tag="u2")
                    nc.scalar.copy(u2[:], h1n[:, (j + k) * 64:(j + k + 1) * 64])
                    tp_ = psC.tile([P, 128], f32, tag="pw2")
                    nc.tensor.transpose(out=tp_[:64, :], in_=u2[:],
                                        identity=ident[:])
                    hl = epi.tile([64, 128], f32, tag="hl")
                    nc.scalar.copy(hl[:], tp_[:64, :])
                    nc.tensor.matmul(out=ps[:, k * 128:(k + 1) * 128],
                                     lhsT=hl[:], rhs=rw2_s[:],
                                     start=True, stop=True)
                nc.vector.tensor_tensor(
                    out=root2[:, j * 128:(j + g) * 128],
                    in0=ps[:, :g * 128], in1=b2_s[:, :g * 128], op=AL.add)
                j += g

        yn = big.tile([P, c.W * 64], f32, tag="yt")
        with nc.named_scope("layer2"):
            layer(ccout[0:c.HALFA, :], ccout[c.HALFA:c.NSTAR, :], root2,
                  yn, 64, w2_s)
        nc.sync.dma_start(yt[:, :].rearrange("(w p) f -> p w f", p=P), yn[:])

    nc.compile()
    return nc


_cache = {}


def prepare(inputs, cfg_kw=None):
    c = Cfg(**(cfg_kw or REAL))
    in_maps, NBA, NBB = _preprocess(c, **inputs)
    key = (tuple(sorted((cfg_kw or REAL).items())), tuple(NBA), tuple(NBB))
    if key not in _cache:
        _cache[key] = _build_program(c, NBA, NBB)
    return c, _cache[key], in_maps


def kernel(x, edge_index, init_w1, root_w1, b1, init_w2, root_w2, b2,
           _trace=False, _cfg=None):
    from concourse import bass_utils
    inputs = dict(x=np.asarray(x), edge_index=np.asarray(edge_index),
                  init_w1=np.asarray(init_w1), root_w1=np.asarray(root_w1),
                  b1=np.asarray(b1), init_w2=np.asarray(init_w2),
                  root_w2=np.asarray(root_w2), b2=np.asarray(b2))
    c, nc, in_maps = prepare(inputs, _cfg)
    res = bass_utils.run_bass_kernel_spmd(
        nc, in_maps, core_ids=list(range(c.NC)), trace=_trace)
    out = np.concatenate([res.results[cc]["yt"] for cc in range(c.NC)],
                         axis=0)[:c.N]
    if _trace:
        kernel._last = res
    return np.ascontiguousarray(out.astype(np.float32))

